# revision 12
# baseline (speedup 1.0000x reference)
"""Sliding context-window gather kernel for Trainium2 (Bass/Tile).

Computes, for x[B=32, T=2000, C=80] and lengths[B]:
    out[b, t, c*11 + i] = x[b, t + i - 5, c]          (zero outside [0, T))
                          * (t < round(T * lengths[b]))
i.e. an 11-tap sliding-window gather along T with channel-major
interleave, masked by per-sample length.

Sharding: pure data-parallel, 4 samples per core across 8 cores.

Per-core layout trick: the host zero-pads x by 5 rows on each side of T,
and each sample is loaded into SBUF as an overlapping-window view
[125 partitions x 26 rows x 80 ch] (one DMA; partition p holds padded
rows 16p .. 16p+26, i.e. t = 16p-5 .. 16p+21). A shift by d in t is then
a single full-width free-dim-offset copy [125, 16, 80] — 11 copies build
the whole interleaved output tile. The length mask is precomputed on
host as a {0,1} float32 [B, T] tensor and applied with 16 per-partition
tensor_scalar multiplies (one per folded row). All HBM traffic is
contiguous-chunk DMA: ~1.04 MB load + 7.04 MB store per sample.
"""

import numpy as np

import concourse.mybir as mybir
from concourse import bacc, bass
from concourse.ap import AP
from concourse.bass_utils import run_bass_kernel_spmd
from concourse.tile import TileContext

LEFT = 5
RIGHT = 5
CTXW = LEFT + RIGHT + 1  # 11
B, T, C = 32, 2000, 80
N_CORES = 8
B_LOC = B // N_CORES  # 4 samples per core
P = 125  # SBUF partitions used per sample fold
Q = 16   # consecutive t rows per partition (P * Q == T)
QG = Q + LEFT + RIGHT  # 26 rows per partition incl. halo
TP = T + LEFT + RIGHT  # padded time length
F32 = mybir.dt.float32

assert P * Q == T


def _build_bass(repeats: int = 1, timing_mode: bool = False):
    nc = bacc.Bacc()
    xp_dram = nc.declare_dram_parameter("xp", [B_LOC, TP, C], F32, isOutput=False)
    msk = nc.declare_dram_parameter("mask", [B_LOC, T], F32, isOutput=False)
    if timing_mode:
        # timing benchmark variant: keep the full-size store traffic on
        # device (internal DRAM buffer) but expose only a tiny external
        # output, so host<->device transfer noise doesn't swamp the
        # marginal-repeat measurement.
        out = nc.dram_tensor("scratch_out", [B_LOC, T, C * CTXW], F32)
        done = nc.declare_dram_parameter("done", [P, 1], F32, isOutput=True)
    else:
        out = nc.declare_dram_parameter(
            "out", [B_LOC, T, C * CTXW], F32, isOutput=True
        )

    QH = Q // 4  # j-rows per chunk
    NCHUNK = Q // QH
    NOBUF = 8  # chunk-output ring length
    ochunk = 0
    with TileContext(nc) as tc:
        with (
            tc.tile_pool(name="xpool", bufs=1) as xpool,
            tc.tile_pool(name="mpool", bufs=1) as mpool,
            tc.tile_pool(name="opool", bufs=1) as opool,
        ):
            for _rep in range(repeats):
                for b in range(B_LOC):
                    X = xpool.tile([P, QG, C], F32, tag=f"X{b % B_LOC}")
                    M = mpool.tile([P, Q], F32, tag=f"M{b % B_LOC}")

                    # overlapping window view: element (p, r, c) reads
                    # x_pad[b, Q*p + r, c]  (rows overlap across partitions)
                    window = AP(
                        xp_dram[b].tensor,
                        b * TP * C,
                        [[Q * C, P], [C, QG], [1, C]],
                    )
                    nc.scalar.dma_start(out=X, in_=window)
                    nc.scalar.dma_start(
                        out=M, in_=msk[b].rearrange("(p q) -> p q", q=Q)
                    )

                    # chunks of QH j-rows each: store each chunk as soon
                    # as its 11 fused shift-mask multiplies finish; chunk
                    # output tiles cycle an explicit NOBUF-deep ring so the
                    # WAR reuse dependency reaches NOBUF chunks back
                    out_b = out[b].rearrange(
                        "(p q) (c i) -> p q c i", q=Q, i=CTXW
                    )
                    for h in range(NCHUNK):
                        j0 = h * QH
                        O = opool.tile(
                            [P, QH, C, CTXW], F32, tag=f"O{ochunk % NOBUF}"
                        )
                        ochunk += 1
                        # O[p, j, c, i] = X[p, j0 + j + i, c] * M[p, j0 + j]
                        m_bcast = M[:, j0 : j0 + QH].unsqueeze(2).broadcast_to(
                            [P, QH, C]
                        )
                        for i in range(CTXW):
                            nc.vector.tensor_mul(
                                out=O[:, :, :, i],
                                in0=X[:, j0 + i : j0 + i + QH, :],
                                in1=m_bcast,
                            )
                        nc.sync.dma_start(
                            out=out_b[:, j0 : j0 + QH], in_=O[:, :, :, :]
                        )
            if timing_mode:
                D = mpool.tile([P, 1], F32, tag="done")
                nc.vector.tensor_copy(out=D, in_=M[:, 0:1])
                nc.sync.dma_start(out=done[:, :], in_=D)
    nc.compile()
    return nc


_NC_CACHE = {}


def _get_nc(repeats: int = 1, timing_mode: bool = False):
    key = (repeats, timing_mode)
    if key not in _NC_CACHE:
        _NC_CACHE[key] = _build_bass(repeats, timing_mode)
    return _NC_CACHE[key]


def _make_in_maps(x, lengths):
    x = np.asarray(x, dtype=np.float32)
    x_pad = np.zeros((B, TP, C), dtype=np.float32)
    x_pad[:, LEFT : LEFT + T, :] = x
    lengths = np.asarray(lengths, dtype=np.float32)
    len_abs = np.round(np.float32(T) * lengths).astype(np.int32)
    mask = (np.arange(T, dtype=np.int32)[None, :] < len_abs[:, None]).astype(
        np.float32
    )  # [B, T]
    return [
        {
            "xp": x_pad[c * B_LOC : (c + 1) * B_LOC],
            "mask": np.ascontiguousarray(mask[c * B_LOC : (c + 1) * B_LOC]),
        }
        for c in range(N_CORES)
    ]


def _run(x, lengths, repeats: int = 1, timing_mode: bool = False, **spmd_kwargs):
    res = run_bass_kernel_spmd(
        _get_nc(repeats, timing_mode),
        _make_in_maps(x, lengths),
        list(range(N_CORES)),
        **spmd_kwargs,
    )
    if timing_mode:
        return None, res
    out = np.concatenate([r["out"] for r in res.results], axis=0)
    return out, res


def kernel(x, lengths):
    out, _ = _run(x, lengths)
    return out
